# revision 33
# baseline (speedup 1.0000x reference)
"""TRN2 Bass kernel for nn_AutoEncoder_14542759264279 (scatter_memory).

Problem (per sample b of 8): scatter-add 500k values into a 128^3 grid by
int coordinates, then total-variation (sum |adjacent diff|) and smoothness
MSE (sum diff^2) losses over the grid, each normalized. Output (2, 8) f32.

Sharding: data-parallel over the batch axis - core b handles sample b
entirely (its own scatter + losses), no cross-core traffic.

Device algorithm per core (PE-matmul scatter; no indirect DMA at all):
  - host groups points by (x0 row, x2 window of 32, x1 window of 32) -
    index metadata only; all FP accumulation stays on device. Each group
    gets at most 2 static 128-point tiles (unified max over the 8 cores);
    the rare excess points go to 1-2 per-row full-width overflow tiles.
  - per 128-point tile, ONE fused DVE is_equal against a replicated iota
    builds both one-hot matrices ([128, 2, 32] bf16, f32 compare inputs):
      S[k, x2rel] = (x2_k == x2rel),  R[k, x1rel] = (x1_k == x1rel)
    Pool multiplies S by v (per-lane broadcast); one PE matmul per tile
    accumulates SV^T @ R into a 32x32 PSUM window of the row's bank
    (explicit tile_position; duplicates accumulate natively in PSUM).
    Overflow tiles use width-128 one-hots over the full bank and run
    first so their start=True initializes all 16 window regions.
  - ACT evacuates each finished row into an SBUF-resident grid stored as
    4 x0-quarter tiles [128(x2), 32(x0), 128(x1)]; the loss phase streams
    per quarter while later quarters still scatter: x0/x1 diffs via
    shifted-AP subtract (Pool, bf16 out) with DVE abs-reduce + ACT Square
    accum_out; x2 (partition) diffs via shift-matrix matmuls with ACT
    Abs/Square accum straight from PSUM; final cross-partition sum via
    ones-matmul.

Measured ~0.68 ms/core on TRN2 (baseline indirect-DMA scatter: 8.4 ms);
limited by DVE/Pool one-hot build throughput under SBUF port contention.

The tile schedule is data-dependent; the program is compiled per schedule
signature and cached (one compile for iid inputs).

Self-contained: hardcodes all shapes; no file reads.
"""
import contextlib
import ctypes
import sys
import types

import numpy as np

P = 128
XS = 128
B = 8
M = 500_000
TV_NORM = float(XS * XS * XS)
MSE_NORM = float(2 * XS * XS - 2 * XS)

_SO_PATH = "/opt/axon/libaxon_pjrt.so"


def _install_ntff_hook():
    """Provide antenv.axon_hooks (NTFF profile hook) if missing."""
    if "antenv.axon_hooks" in sys.modules:
        return
    try:
        import antenv
    except ImportError:
        return

    def _make_hook():
        try:
            lib = ctypes.CDLL(_SO_PATH)
        except OSError:
            return None
        if not hasattr(lib, "axon_start_nrt_profile"):
            return None
        lib.axon_start_nrt_profile.argtypes = [
            ctypes.POINTER(ctypes.c_int64),
            ctypes.c_size_t,
        ]
        lib.axon_start_nrt_profile.restype = ctypes.c_int64
        lib.axon_stop_nrt_profile.argtypes = [ctypes.c_char_p]
        lib.axon_stop_nrt_profile.restype = ctypes.c_int64

        @contextlib.contextmanager
        def _hook(output_dir, device_ids):
            import jax

            jax.devices()
            if device_ids:
                ids = (ctypes.c_int64 * len(device_ids))(*device_ids)
                rc = lib.axon_start_nrt_profile(ids, len(device_ids))
            else:
                rc = lib.axon_start_nrt_profile(None, 0)
            if rc != 0:
                raise RuntimeError(f"axon_start_nrt_profile rc={rc}")
            try:
                yield
            finally:
                n = lib.axon_stop_nrt_profile(str(output_dir).encode())
                print(f"ntff profile: {n} file(s) in {output_dir}", file=sys.stderr)

        return _hook

    mod = types.ModuleType("antenv.axon_hooks")
    mod._hook = _make_hook()
    mod.get_axon_ntff_profile_hook = lambda: mod._hook

    def _set(h):
        mod._hook = h

    mod.set_axon_ntff_profile_hook = _set
    sys.modules["antenv.axon_hooks"] = mod
    antenv.axon_hooks = mod


def _split_waits(nc, mybir):
    """walrus here allows only 1 sem wait per instruction; hoist extras
    onto preceding same-engine NoOps."""
    n = 0
    for f in nc.m.functions:
        for bb in f.blocks:
            il = bb.instructions
            i = 0
            while i < len(il):
                inst = il[i]
                si = inst.sync_info
                if si is not None and len(si.on_wait) > 1:
                    waits = list(si.on_wait)
                    si.on_wait = waits[:1]
                    pre = []
                    for w in waits[1:]:
                        nop = mybir.InstNoOp(name=f"I-waitsplit-{n}", ins=[], outs=[])
                        n += 1
                        nop.engine = inst.engine
                        nop.sync_info = mybir.SyncInfo(on_wait=[w], on_update=[])
                        pre.append(nop)
                    il[i:i] = pre
                    i += len(pre)
                i += 1
    return n


def _patch_tile_drain(tile, bass_rust, mybir):
    """Split the tail-drain waits (same 1-wait-per-instruction limit)."""

    def _drain_and_barrier(self, tick_clock, wait_clock):
        drain_inst = self.nc.sync.drain()
        wait_clock.add_sem_waits(
            drain_inst.ins, bass_rust.ScopedClock({None: tick_clock.global_clock})
        )
        si = drain_inst.ins.sync_info
        waits = list(si.on_wait) if si is not None else []
        if len(waits) > 1:
            si.on_wait = waits[:1]
            for i in range(1, len(waits)):
                extra = self.nc.sync.drain()
                esi = extra.ins.sync_info
                if esi is None:
                    extra.ins.sync_info = mybir.SyncInfo(
                        on_wait=[waits[i]], on_update=[]
                    )
                else:
                    esi.on_wait = [waits[i]]
        self.nc.all_engine_barrier()
        assert self.sems is not None
        popped = self.nc._tile_sem_poison_stack.pop()
        assert popped is self._sem_poison
        sems = sorted(
            s.num if hasattr(s, "num") else s
            for s in self.sems.allocated().values()
        )
        for i in range(0, len(sems), 4):
            self.nc.clear_and_free_semaphores(sems[i : i + 4])
        self.nc.all_engine_barrier()

    tile.TileContext._drain_and_barrier = _drain_and_barrier


W2 = 32          # x2 window (stationary one-hot width)
W1 = 32          # x1 window (moving one-hot width)
NX2W = XS // W2  # 4
NX1W = XS // W1  # 4
NGRP = NX2W * NX1W  # 16 groups per x0 row
GB = 32          # one-hot build batch (tiles per DVE/Pool instruction)


def build_program(gtiles):
    """gtiles: tuple of 128*NGRP ints, tiles per (x0 row, x2win, x1win)
    group (shared across cores)."""
    import concourse.bass as bass
    import concourse.mybir as mybir
    import concourse.tile as tile
    import bass_rust

    _patch_tile_drain(tile, bass_rust, mybir)

    f32 = mybir.dt.float32
    bf16 = mybir.dt.bfloat16
    Alu = mybir.AluOpType
    Act = mybir.ActivationFunctionType

    T = int(sum(gtiles))
    nc = bass.Bass("TRN2", target_bir_lowering=False, debug=False)
    xx_d = nc.dram_tensor("xx", [P, 2 * T], f32, kind="ExternalInput")
    v_d = nc.dram_tensor("vc", [P, T], bf16, kind="ExternalInput")
    iota_d = nc.dram_tensor("iota", [P, GB, 2, W2], f32, kind="ExternalInput")
    iotaf_d = nc.dram_tensor("iotaf", [P, 1, 2, P], f32, kind="ExternalInput")
    sdiff_d = nc.dram_tensor("sdiff", [P, P], f32, kind="ExternalInput")
    out_d = nc.dram_tensor("out", [1, 2], f32, kind="ExternalOutput")

    NS = 44

    with tile.TileContext(nc) as tc:
        with tc.tile_pool(name="setup", bufs=1) as sp:
            iota_t = sp.tile([P, GB, 2, W2], f32)
            iotaf_t = sp.tile([P, 1, 2, P], f32)
            sdiff_t = sp.tile([P, P], f32)
            nc.sync.dma_start(out=iota_t[:], in_=iota_d.ap()[:])
            nc.sync.dma_start(out=iotaf_t[:], in_=iotaf_d.ap()[:])
            nc.sync.dma_start(out=sdiff_t[:], in_=sdiff_d.ap()[:])
            xx = sp.tile([P, 2 * T], f32, tag="xx")
            vc = sp.tile([P, T], bf16, tag="vc")
            nc.sync.dma_start(out=xx[:], in_=xx_d.ap()[:])
            nc.sync.dma_start(out=vc[:], in_=v_d.ap()[:])

            # grid in x0-quarters: [x2, x0q, x1] so the loss phase can
            # stream while later quarters are still scattering
            gq = [
                sp.tile([P, 32, XS], f32, tag=f"gq{q}", name=f"gq{q}")
                for q in range(4)
            ]

            tvp = sp.tile([P, NS], f32, tag="tvp")
            msep = sp.tile([P, NS], f32, tag="msep")
            nc.vector.memset(tvp[:], 0.0)
            nc.vector.memset(msep[:], 0.0)

            with tc.tile_pool(name="sp_F", bufs=4) as poolF, \
                 tc.tile_pool(name="sp_SV", bufs=4) as poolSV, \
                 tc.tile_pool(name="dtmp", bufs=2) as dt, \
                 tc.tile_pool(name="d3sq", bufs=4) as sqb, \
                 tc.tile_pool(name="acc_ps", bufs=4, space="PSUM") as aps, \
                 tc.tile_pool(name="d3ps", bufs=2, space="PSUM") as dps, \
                 tc.tile_pool(name="fin_ps", bufs=1, space="PSUM") as fps:

                def loss_quarter(q):
                    """emit loss ops for grid quarter q (rows 32q..32q+31)"""
                    G = gq[q]
                    # d1 within quarter (31 pairs) + boundary pair to prev
                    D1 = dt.tile([P, 32, XS], bf16, tag="D")
                    nc.vector.tensor_tensor(
                        out=D1[:, :31, :], in0=G[:, 1:32, :],
                        in1=G[:, 0:31, :], op=Alu.subtract)
                    if q > 0:
                        nc.vector.tensor_tensor(
                            out=D1[:, 31:32, :], in0=G[:, 0:1, :],
                            in1=gq[q - 1][:, 31:32, :], op=Alu.subtract)
                    n1 = 32 if q > 0 else 31
                    nc.scalar.activation(
                        out=D1[:, :n1, :], in_=D1[:, :n1, :], func=Act.Abs,
                        accum_out=tvp[:, q : q + 1])
                    nc.scalar.activation(
                        out=D1[:, :n1, :], in_=D1[:, :n1, :], func=Act.Square,
                        accum_out=msep[:, q : q + 1])
                    # d2 within rows (alternate engines to balance load)
                    D2 = dt.tile([P, 32, XS], bf16, tag="D")
                    nc.vector.tensor_tensor(
                        out=D2[:, :, : XS - 1], in0=G[:, :, 1:],
                        in1=G[:, :, : XS - 1], op=Alu.subtract)
                    nc.scalar.activation(
                        out=D2[:, :, : XS - 1], in_=D2[:, :, : XS - 1],
                        func=Act.Abs, accum_out=tvp[:, 4 + q : 5 + q])
                    nc.scalar.activation(
                        out=D2[:, :, : XS - 1], in_=D2[:, :, : XS - 1],
                        func=Act.Square, accum_out=msep[:, 4 + q : 5 + q])
                    # d3: partition-adjacent via shift-matrix matmul
                    for m in range(8):
                        h = q * 8 + m
                        d3p = dps.tile([P, 4 * XS], f32, space="PSUM", tag="d3")
                        nc.tensor.matmul(
                            out=d3p[:], lhsT=sdiff_t[:],
                            rhs=G[:, 4 * m : 4 * m + 4, :],
                            start=True, stop=True)
                        sqa = sqb.tile([P, 4 * XS], bf16, tag="sqa")
                        nc.scalar.activation(
                            out=sqa[:], in_=d3p[:], func=Act.Abs,
                            accum_out=tvp[:, 8 + h : 9 + h])
                        sqm = sqb.tile([P, 4 * XS], bf16, tag="sqm")
                        nc.scalar.activation(
                            out=sqm[:], in_=d3p[:], func=Act.Square,
                            accum_out=msep[:, 8 + h : 9 + h])

                # ---- Phase A: PE scatter, loss streamed per quarter ----
                # gtiles[:P*NGRP] = window tiles (capped at 2, W=32 regions);
                # gtiles[P*NGRP:] = per-row full-width overflow tile counts
                col = 0
                for p in range(P):
                    novf = gtiles[P * NGRP + p]
                    ps = aps.tile([P, P], f32, space="PSUM", tag="acc")
                    # full-width overflow tiles first: the initial one
                    # resets the whole bank, window chains then accumulate
                    for t in range(novf):
                        c0 = col + t
                        Fo = poolF.tile([P, 1, 2, P], bf16, tag="Fo")
                        nc.vector.tensor_tensor(
                            out=Fo[:], in0=iotaf_t[:, :1, :, :],
                            in1=xx[:, 2 * c0 : 2 * c0 + 2]
                                .to_broadcast([P, 2, P]),
                            op=Alu.is_equal)
                        SVo = poolSV.tile([P, 1, P], bf16, tag="SVo")
                        nc.gpsimd.tensor_tensor(
                            out=SVo[:], in0=Fo[:, :1, 0, :],
                            in1=vc[:, c0 : c0 + 1].to_broadcast([P, 1, P]),
                            op=Alu.mult)
                        nc.tensor.matmul(
                            out=ps[:], lhsT=SVo[:, 0, :], rhs=Fo[:, 0, 1, :],
                            start=(t == 0), stop=False,
                            tile_position=(0, 0), skip_group_check=True)
                    col += novf
                    wdescs = []
                    for j in range(NGRP):
                        nt = gtiles[p * NGRP + j]
                        for t in range(nt):
                            wdescs.append((j, t == 0, t == nt - 1))
                    b = 0
                    while b < len(wdescs):
                        g = min(GB, len(wdescs) - b)
                        c0 = col + b
                        F = poolF.tile([P, GB, 2, W2], bf16, tag="F")
                        nc.vector.tensor_tensor(
                            out=F[:, :g, :, :],
                            in0=iota_t[:, 0:1, :, :].to_broadcast(
                                [P, g, 2, W2]),
                            in1=xx[:, 2 * c0 : 2 * (c0 + g)]
                                .to_broadcast([P, 2 * g, W2]),
                            op=Alu.is_equal)
                        SV = poolSV.tile([P, GB, W2], bf16, tag="SV")
                        nc.gpsimd.tensor_tensor(
                            out=SV[:, :g, :], in0=F[:, :g, 0, :],
                            in1=vc[:, c0 : c0 + g].to_broadcast([P, g, W2]),
                            op=Alu.mult)
                        for k in range(g):
                            j, st, sp_ = wdescs[b + k]
                            x2b = (j // NX1W) * W2
                            x1b = (j % NX1W) * W1
                            nc.tensor.matmul(
                                out=ps[x2b : x2b + W2, x1b : x1b + W1],
                                lhsT=SV[:, k, :],
                                rhs=F[:, k, 1, :],
                                start=(st and novf == 0), stop=sp_,
                                tile_position=(0, x2b),
                                skip_group_check=True)
                        b += g
                    col += len(wdescs)
                    nc.scalar.copy(out=gq[p // 32][:, p % 32, :], in_=ps[:])
                    if p % 32 == 31:
                        loss_quarter(p // 32)

                # ---- final: reduce slots, cross-partition sum, scale ----
                tvcol = sp.tile([P, 1], f32)
                msecol = sp.tile([P, 1], f32)
                nc.vector.tensor_reduce(
                    out=tvcol[:], in_=tvp[:, :NS],
                    axis=mybir.AxisListType.X, op=Alu.add)
                nc.vector.tensor_reduce(
                    out=msecol[:], in_=msep[:, :NS],
                    axis=mybir.AxisListType.X, op=Alu.add)
                ones = sp.tile([P, 1], f32)
                nc.vector.memset(ones[:], 1.0)
                tv_ps = fps.tile([1, 1], f32, space="PSUM", tag="fin")
                nc.tensor.matmul(out=tv_ps[:], lhsT=tvcol[:], rhs=ones[:],
                                 start=True, stop=True)
                mse_ps = fps.tile([1, 1], f32, space="PSUM", tag="fin2")
                nc.tensor.matmul(out=mse_ps[:], lhsT=msecol[:], rhs=ones[:],
                                 start=True, stop=True)
                res = sp.tile([1, 2], f32)
                nc.scalar.mul(out=res[:, 0:1], in_=tv_ps[:], mul=1.0 / TV_NORM)
                nc.scalar.mul(out=res[:, 1:2], in_=mse_ps[:], mul=1.0 / MSE_NORM)
                nc.sync.dma_start(out=out_d.ap()[:], in_=res[:])

    _split_waits(nc, mybir)
    return nc


_PROG_CACHE = {}


def _get_program(tpr):
    key = tuple(tpr)
    if key not in _PROG_CACHE:
        _PROG_CACHE[key] = build_program(key)
    return _PROG_CACHE[key]


def _host_constants():
    # iota over the window width, replicated (GB, 2) times: compare target
    # alternates x2 (S half) / x1 (R half) per tile
    iota = np.broadcast_to(
        np.arange(W2, dtype=np.float32)[None, None, None, :], (P, GB, 2, W2)
    ).astype(np.float32)
    iotaf = np.broadcast_to(
        np.arange(P, dtype=np.float32)[None, None, None, :], (P, 1, 2, P)
    ).astype(np.float32)
    # sdiff[k, m] = +1 if k==m+1 else -1 if k==m (column 127 zeroed)
    sdiff = np.zeros((P, P), np.float32)
    for m in range(P - 1):
        sdiff[m + 1, m] = 1.0
        sdiff[m, m] = -1.0
    return np.ascontiguousarray(iota), np.ascontiguousarray(iotaf), sdiff


def _pack_core(idx, val, tiles_w, gbase, obase, T):
    """Pack one sample's points into the tile-column layout: capped window
    tiles (relative coords) + per-row full-width overflow tiles (raw
    coords). Returns xx [P, 2T] f32, vc [P, T] bf16."""
    import ml_dtypes

    x0 = idx[:, 0].astype(np.int64)
    x1 = idx[:, 1].astype(np.int64)
    x2 = idx[:, 2].astype(np.int64)
    gkey = x0 * NGRP + (x2 // W2) * NX1W + (x1 // W1)
    order = np.argsort(gkey, kind="stable")
    gs = gkey[order]
    x1o = x1[order]
    x2o = x2[order]
    vs = val[order].astype(np.float32)

    counts = np.bincount(gs, minlength=P * NGRP)
    grp_start = np.concatenate([[0], np.cumsum(counts)[:-1]])
    within = np.arange(len(gs)) - grp_start[gs]
    cap = tiles_w[gs] * P
    is_ovf = within >= cap

    xx = np.zeros((P, 2 * T), np.float32)
    vc = np.zeros((P, T), np.float32)

    wsel = ~is_ovf
    lane = within[wsel] % P
    colpos = gbase[gs[wsel]] + within[wsel] // P
    xx[lane, 2 * colpos] = (x2o[wsel] % W2).astype(np.float32)
    xx[lane, 2 * colpos + 1] = (x1o[wsel] % W1).astype(np.float32)
    vc[lane, colpos] = vs[wsel]

    o_rows = gs[is_ovf] // NGRP
    cnt = np.bincount(o_rows, minlength=P)
    st = np.concatenate([[0], np.cumsum(cnt)[:-1]])
    rank = np.arange(len(o_rows)) - st[o_rows]
    olane = rank % P
    ocol = obase[o_rows] + rank // P
    xx[olane, 2 * ocol] = x2o[is_ovf].astype(np.float32)
    xx[olane, 2 * ocol + 1] = x1o[is_ovf].astype(np.float32)
    vc[olane, ocol] = vs[is_ovf]
    return (xx, vc.astype(ml_dtypes.bfloat16))


def kernel(indices, values, xsize):
    sys.path.insert(0, "/opt/trn_rl_repo")
    _install_ntff_hook()
    from concourse import bass_utils

    indices = np.asarray(indices, dtype=np.int32)
    values = np.asarray(values, dtype=np.float32)
    assert int(xsize) == XS
    assert indices.shape == (B, M, 3) and values.shape == (B, M)

    # unified static tile schedule: per (x0, x2win, x1win) group, window
    # tiles capped at 2 (max over cores, >=1 so every PSUM region gets
    # initialized); excess points go to per-row full-width overflow tiles
    counts = np.zeros((B, P * NGRP), np.int64)
    for b in range(B):
        gkey = (indices[b, :, 0].astype(np.int64) * NGRP
                + (indices[b, :, 2].astype(np.int64) // W2) * NX1W
                + (indices[b, :, 1].astype(np.int64) // W1))
        counts[b] = np.bincount(gkey, minlength=P * NGRP)
    tiles_w = np.clip(
        np.ceil(counts / P).astype(np.int64).max(axis=0), 1, 2
    )
    ovf = np.maximum(counts - tiles_w[None, :] * P, 0)
    ovf_row = ovf.reshape(B, P, NGRP).sum(axis=2)
    novf = np.ceil(ovf_row / P).astype(np.int64).max(axis=0)

    # column layout: per row p, overflow tiles first, then window tiles
    row_w = tiles_w.reshape(P, NGRP).sum(axis=1)
    row_total = row_w + novf
    row_base = np.concatenate([[0], np.cumsum(row_total)[:-1]])
    obase = row_base.copy()
    gbase = np.zeros(P * NGRP, np.int64)
    for p_ in range(P):
        gb = row_base[p_] + novf[p_] + np.concatenate(
            [[0], np.cumsum(tiles_w[p_ * NGRP : (p_ + 1) * NGRP])[:-1]]
        )
        gbase[p_ * NGRP : (p_ + 1) * NGRP] = gb
    T = int(row_total.sum())
    gtiles = tuple(int(x) for x in tiles_w) + tuple(int(x) for x in novf)

    iota, iotaf, sdiff = _host_constants()
    in_maps = []
    for b in range(B):
        xx, vc = _pack_core(indices[b], values[b], tiles_w, gbase, obase, T)
        in_maps.append(
            {"xx": xx, "vc": vc, "iota": iota, "iotaf": iotaf,
             "sdiff": sdiff}
        )

    nc = _get_program(gtiles)
    import os

    trace = bool(os.environ.get("TRNK_TRACE"))
    res = bass_utils.run_bass_kernel_spmd(
        nc, in_maps, core_ids=list(range(B)), trace=trace
    )
    if trace and res.exec_time_ns is not None:
        print(f"HW exec time: {res.exec_time_ns} ns")
    tv = np.array([res.results[b]["out"][0, 0] for b in range(B)], np.float32)
    mse = np.array([res.results[b]["out"][0, 1] for b in range(B)], np.float32)
    return np.stack([tv, mse]).astype(np.float32)


if __name__ == "__main__":
    rng = np.random.default_rng(0)
    idx = rng.integers(0, XS, (B, M, 3), dtype=np.int32)
    val = rng.standard_normal((B, M), dtype=np.float32)
    out = kernel(idx, val, XS)
    print(out)
